# revision 1
# baseline (speedup 1.0000x reference)
"""DeepSeekMoE layer on 8 Trainium2 NeuronCores.

Problem (hardcoded): B=2, T=1024, C=1024, H=4096, E_routed=8 (top-2 sigmoid
gating), E_shared=2, fp32.

Sharding: 2-way expert-parallel x 4-way token-parallel.
  Core c (g = c//4, q = c%4) processes token quarter q (512 tokens) for the
  expert set {shared[g], routed[4g], .., routed[4g+3]} and emits the partial
  sum of those 5 expert contributions in transposed layout [C, 512].
  Host: out[q] = partial[q] + partial[q+4] + u[q]  (residual on host).

Device kernel (per core, SPMD — identical program, different data):
  T-layout throughout: activations [C-partition, token-free], tokens N=512.
  rmsnorm stats via squared tiles + ones-matmul column reduce; top-2 sigmoid
  gating in token-layout, transposed via PE, broadcast via one-hot matmuls.
  Expert MLP: W1 stationary [128,128] tiles x xnT moving (N=512) -> PSUM ->
  gelu(+b1) on ACT -> h_act (f32r) -> W2 stationary x h_act moving -> PSUM
  (with b2 folded in via a K=1 matmul) -> eviction (gate mult + add) on DVE
  into the SBUF accumulator. Matmuls run in float32r (TF32-like, full PE rate
  at N>=512).

loop_m > 1 wraps the whole body in a hardware For_i loop — used only for
wall-clock timing (difference M vs 1 iterations to cancel dispatch overhead).
"""
import contextlib
import os
import numpy as np

import concourse.bass as bass
import concourse.tile as tile
from concourse import bacc, mybir
from concourse import bass_utils
from concourse.alu_op_type import AluOpType
from concourse.masks import make_identity

F32 = mybir.dt.float32
F32R = mybir.dt.float32r
AF = mybir.ActivationFunctionType
AX = mybir.AxisListType

B, T, C, H = 2, 1024, 1024, 4096
E_R, E_S = 8, 2
NCORES = 8
TOKC = 512          # tokens per core
CK = C // 128       # 8 c-tiles
HK = H // 128       # 32 h-tiles
NMC = 16            # W1 m-chunks (each 2 h-tiles = 256 h cols)
NE = 5              # expert passes per core: 1 shared + 4 routed
EPS = 1.1920929e-07
NEG = -1e30

_CACHE = {}


def _build_program(loop_m=1):
    nc = bacc.Bacc("TRN2", target_bir_lowering=False, debug=False,
                   enable_asserts=False)

    d = {}
    d["uT"] = nc.dram_tensor("uT", [128, CK * TOKC], F32, kind="ExternalInput").ap()
    d["uTr"] = nc.dram_tensor("uTr", [128, CK * TOKC], F32R, kind="ExternalInput").ap()
    d["w1"] = nc.dram_tensor("w1", [NE, NMC, 128, CK * 256], F32R, kind="ExternalInput").ap()
    d["w2"] = nc.dram_tensor("w2", [NE, HK // 2, 128, 2 * 1024], F32R, kind="ExternalInput").ap()
    d["b1"] = nc.dram_tensor("b1t", [128, NE * HK], F32, kind="ExternalInput").ap()
    d["b2r"] = nc.dram_tensor("b2r", [1, NE * CK * 128], F32R, kind="ExternalInput").ap()
    d["cen"] = nc.dram_tensor("cen", [128, CK * E_R], F32R, kind="ExternalInput").ap()
    d["esel"] = nc.dram_tensor("esel", [E_R, 4 * 128], F32R, kind="ExternalInput").ap()
    d["out"] = nc.dram_tensor("outT", [128, CK * TOKC], F32, kind="ExternalOutput").ap()

    with tile.TileContext(nc) as tc:
        with (
            tc.tile_pool(name="cst", bufs=1) as cst,
            tc.tile_pool(name="io", bufs=1) as io,
            tc.tile_pool(name="wk", bufs=1) as wk,
            tc.tile_pool(name="w1p", bufs=2) as w1p,
            tc.tile_pool(name="w2p", bufs=3) as w2p,
            tc.tile_pool(name="hp", bufs=32) as hp,
            tc.tile_pool(name="pp", bufs=8, space="PSUM") as pp,
        ):
            pools = dict(cst=cst, io=io, wk=wk, w1p=w1p, w2p=w2p, hp=hp, pp=pp)
            loop = tc.For_i(0, loop_m, 1) if loop_m > 1 else contextlib.nullcontext()
            with loop:
                _moe_body(nc, d, pools)

    nc.compile()
    return nc


def _moe_body(nc, d, p):
    cst, io, wk, w1p, w2p, hp, pp = (
        p["cst"], p["io"], p["wk"], p["w1p"], p["w2p"], p["hp"], p["pp"])

    # ---- constants ----
    ident = cst.tile([128, 128], F32, tag="ident", name="ident")
    make_identity(nc, ident[:])
    ones_f = cst.tile([128, 1], F32, tag="ones_f", name="ones_f")
    nc.gpsimd.memset(ones_f[:], 1.0)
    onescol = cst.tile([128, 1], F32R, tag="onescol", name="onescol")
    nc.vector.tensor_copy(onescol[:], ones_f[:])
    ones512_f = cst.tile([1, TOKC], F32, tag="ones512_f", name="ones512_f")
    nc.gpsimd.memset(ones512_f[:], 1.0)
    ones512 = cst.tile([1, TOKC], F32R, tag="ones512", name="ones512")
    nc.vector.tensor_copy(ones512[:], ones512_f[:])
    onesrow_f = cst.tile([1, 128], F32, tag="onesrow_f", name="onesrow_f")
    nc.gpsimd.memset(onesrow_f[:], 1.0)
    onesrow = cst.tile([1, 128], F32R, tag="onesrow", name="onesrow")
    nc.vector.tensor_copy(onesrow[:], onesrow_f[:])
    epsb = cst.tile([1, 1], F32, tag="epsb", name="epsb")
    nc.gpsimd.memset(epsb[:], EPS)
    sclb = cst.tile([1, 1], F32, tag="sclb", name="sclb")
    nc.gpsimd.memset(sclb[:], 1.0 / C)
    negb = cst.tile([128, 1], F32, tag="negb", name="negb")
    nc.gpsimd.memset(negb[:], NEG)

    # ---- input loads (single DMA each) ----
    uT = io.tile([128, CK * TOKC], F32, tag="uT", name="uT")
    uTr = io.tile([128, CK * TOKC], F32R, tag="uTr", name="uTr")
    for k in range(CK):
        sl = slice(TOKC * k, TOKC * (k + 1))
        nc.sync.dma_start(uT[:, sl], d["uT"][:, sl])
        nc.sync.dma_start(uTr[:, sl], d["uTr"][:, sl])
    cen = io.tile([128, CK * E_R], F32R, tag="cen", name="cen")
    nc.sync.dma_start(cen[:], d["cen"])
    esel = io.tile([E_R, 4 * 128], F32R, tag="esel", name="esel")
    nc.sync.dma_start(esel[:], d["esel"])
    b1 = io.tile([128, NE * HK], F32, tag="b1", name="b1")
    nc.sync.dma_start(b1[:], d["b1"])

    def uslc(k):
        return slice(TOKC * k, TOKC * (k + 1))

    # ---- rmsnorm stats: invrms over all 512 tokens ----
    ss_ps = pp.tile([1, TOKC], F32, tag="pp", name="ss_ps")
    for k in range(CK):
        usq = wk.tile([128, TOKC], F32R, tag="usq", bufs=2, name=f"usq{k}")
        nc.vector.tensor_mul(usq[:], uT[:, uslc(k)], uT[:, uslc(k)])
        nc.tensor.matmul(ss_ps[:], onescol[:], usq[:],
                         start=(k == 0), stop=(k == CK - 1))
    rms = wk.tile([1, TOKC], F32, tag="rms", name="rms")
    nc.scalar.activation(rms[:], ss_ps[:], AF.Sqrt, bias=epsb[:], scale=sclb[:])
    invr = wk.tile([1, TOKC], F32R, tag="invr", name="invr")
    with nc.allow_low_precision(reason="invrms feeds a f32r matmul"):
        nc.vector.reciprocal(invr[:], rms[:])

    # ---- normalized activations: xn[k] = uT[k] * invrep (g folded into W1
    # on the host, so shared/routed share one normalized activation set) ----
    ir_ps = pp.tile([128, TOKC], F32, tag="pp", name="ir_ps")
    nc.tensor.matmul(ir_ps[:], onesrow[:], invr[:], start=True, stop=True)
    xns = []
    for k in range(CK):
        xs = io.tile([128, TOKC], F32R, tag=f"xns{k}", name=f"xns{k}")
        nc.vector.tensor_tensor(xs[:], uT[:, uslc(k)], ir_ps[:], AluOpType.mult)
        xns.append(xs)

    # ---- top-2 sigmoid gating ----
    gT = wk.tile([E_R, TOKC], F32R, tag="gT", name="gT")
    for tt in range(TOKC // 128):
        sc_ps = pp.tile([128, E_R], F32, tag="pp", name=f"sc_ps{tt}")
        for k in range(CK):
            nc.tensor.matmul(
                sc_ps[:], uTr[:, TOKC * k + 128 * tt:TOKC * k + 128 * (tt + 1)],
                cen[:, E_R * k:E_R * (k + 1)],
                start=(k == 0), stop=(k == CK - 1))
        sig = wk.tile([128, E_R], F32, tag="sig", bufs=2, name=f"sig{tt}")
        den = wk.tile([128, 1], F32, tag="den", bufs=2, name=f"den{tt}")
        nc.scalar.activation(sig[:], sc_ps[:], AF.Sigmoid, accum_out=den[:])
        invd = wk.tile([128, 1], F32, tag="invd", bufs=2, name=f"invd{tt}")
        nc.vector.reciprocal(invd[:], den[:])
        m1 = wk.tile([128, 1], F32, tag="m1", bufs=2, name=f"m1_{tt}")
        nc.vector.reduce_max(m1[:], sig[:], axis=AX.X)
        mk1 = wk.tile([128, E_R], F32, tag="mk1", bufs=2, name=f"mk1_{tt}")
        nc.vector.tensor_scalar(mk1[:], sig[:], m1[:], None, AluOpType.is_ge)
        s2 = wk.tile([128, E_R], F32, tag="s2", bufs=2, name=f"s2_{tt}")
        nc.vector.scalar_tensor_tensor(s2[:], mk1[:], negb[:], sig[:],
                                       AluOpType.mult, AluOpType.add)
        m2 = wk.tile([128, 1], F32, tag="m2", bufs=2, name=f"m2_{tt}")
        nc.vector.reduce_max(m2[:], s2[:], axis=AX.X)
        mk = wk.tile([128, E_R], F32, tag="mk", bufs=2, name=f"mk_{tt}")
        nc.vector.tensor_scalar(mk[:], sig[:], m2[:], None, AluOpType.is_ge)
        gsel = wk.tile([128, E_R], F32, tag="gsel", bufs=2, name=f"gsel{tt}")
        nc.vector.tensor_mul(gsel[:], sig[:], mk[:])
        gt8 = wk.tile([128, E_R], F32, tag="gt8", bufs=2, name=f"gt8_{tt}")
        nc.vector.tensor_scalar_mul(gt8[:], gsel[:], invd[:])
        tr_ps = pp.tile([E_R, 128], F32, tag="pp", name=f"tr_ps{tt}")
        nc.tensor.transpose(tr_ps[:], gt8[:], ident[:])
        nc.vector.tensor_copy(gT[:, 128 * tt:128 * (tt + 1)], tr_ps[:])

    # select + broadcast this core's routed-expert gates: one matmul per
    # expert with a one-hot row matrix [E_R, 128] as the stationary side.
    wrep = []
    for j in range(4):
        wr_ps = pp.tile([128, TOKC], F32, tag="pp", name=f"wr_ps{j}")
        nc.tensor.matmul(wr_ps[:], esel[:, 128 * j:128 * (j + 1)], gT[:],
                         start=True, stop=True)
        wr = io.tile([128, TOKC], F32, tag=f"wrep{j}", name=f"wrep{j}")
        nc.vector.tensor_copy(wr[:], wr_ps[:])
        wrep.append(wr)

    # ---- accumulator (one tile, c-tile slices) ----
    acc = io.tile([128, CK * TOKC], F32, tag="acc", name="acc")

    # ---- expert passes ----
    for e in range(NE):
        xn = xns
        b2e = wk.tile([1, CK * 128], F32R, tag="b2e", bufs=2, name=f"b2e{e}")
        nc.sync.dma_start(b2e[:], d["b2r"][:, e * CK * 128:(e + 1) * CK * 128])
        h_act = []
        for mc in range(NMC):
            w1c = w1p.tile([128, CK * 256], F32R, tag="w1c", name=f"w1c_{e}_{mc}")
            half = CK * 256 // 2
            nc.sync.dma_start(w1c[:, :half], d["w1"][e, mc][:, :half])
            nc.sync.dma_start(w1c[:, half:], d["w1"][e, mc][:, half:])
            ph = [pp.tile([128, TOKC], F32, tag="pp", name=f"ph_{e}_{mc}_{m}")
                  for m in range(2)]
            for k in range(CK):
                for m in range(2):
                    nc.tensor.matmul(
                        ph[m][:], w1c[:, 256 * k + 128 * m:256 * k + 128 * (m + 1)],
                        xn[k][:], start=(k == 0), stop=(k == CK - 1))
            for m in range(2):
                hh = 2 * mc + m
                ht = hp.tile([128, TOKC], F32R, tag="h", name=f"h_{e}_{hh}")
                nc.scalar.activation(ht[:], ph[m][:], AF.Gelu,
                                     bias=b1[:, e * HK + hh:e * HK + hh + 1])
                h_act.append(ht)
        py = [pp.tile([128, TOKC], F32, tag="pp", name=f"py_{e}_{m}")
              for m in range(CK)]
        # b2 bias seeds each accumulation group via a K=1 one-hot matmul
        for m in range(CK):
            nc.tensor.matmul(py[m][:], b2e[:, m * 128:(m + 1) * 128], ones512[:],
                             start=True, stop=False)
        for kk in range(HK // 2):
            w2s = w2p.tile([128, 2 * 1024], F32R, tag="w2s", name=f"w2s_{e}_{kk}")
            nc.sync.dma_start(w2s[:, :1024], d["w2"][e, kk][:, :1024])
            nc.sync.dma_start(w2s[:, 1024:], d["w2"][e, kk][:, 1024:])
            for k2 in range(2):
                for m in range(CK):
                    nc.tensor.matmul(
                        py[m][:],
                        w2s[:, 1024 * k2 + 128 * m:1024 * k2 + 128 * (m + 1)],
                        h_act[2 * kk + k2][:], start=False,
                        stop=(kk == HK // 2 - 1 and k2 == 1))
        for m in range(CK):
            aslc = acc[:, uslc(m)]
            if e == 0:
                nc.vector.tensor_copy(aslc, py[m][:])
            else:
                nc.vector.tensor_tensor(py[m][:], py[m][:], wrep[e - 1][:],
                                        AluOpType.mult)
                nc.vector.tensor_add(aslc, aslc, py[m][:])

    # ---- store (single DMA) ----
    nc.sync.dma_start(d["out"], acc[:])


def _prep_inputs(u, g_shared, W1_s, b1_s, W2_s, b2_s,
                 g_routed, W1_r, b1_r, W2_r, b2_r, centroids):
    f = np.float32
    u2 = np.ascontiguousarray(np.asarray(u, f).reshape(B * T, C))
    cenT = np.ascontiguousarray(
        np.asarray(centroids, f).reshape(CK, 128, E_R).transpose(1, 0, 2)
    ).reshape(128, CK * E_R)
    gsh = np.asarray(g_shared, f).reshape(C, 1)
    grt = np.asarray(g_routed, f).reshape(C, 1)

    in_maps = []
    group_cache = {}
    for c in range(NCORES):
        g, q = c // 4, c % 4
        if g not in group_cache:
            W1c = np.concatenate(
                [np.asarray(W1_s[g:g + 1], f) * gsh[None],
                 np.asarray(W1_r[4 * g:4 * g + 4], f) * grt[None]], axis=0)
            W2c = np.concatenate([np.asarray(W2_s[g:g + 1], f),
                                  np.asarray(W2_r[4 * g:4 * g + 4], f)], axis=0)
            b1c = np.concatenate([np.asarray(b1_s[g:g + 1], f),
                                  np.asarray(b1_r[4 * g:4 * g + 4], f)], axis=0)
            b2c = np.concatenate([np.asarray(b2_s[g:g + 1], f),
                                  np.asarray(b2_r[4 * g:4 * g + 4], f)], axis=0)
            # [NE, NMC, 128, CK*256]: W1c[e][128k+p, 256mc+j] -> [e, mc, p, (k j)]
            w1h = np.ascontiguousarray(
                W1c.reshape(NE, CK, 128, NMC, 256).transpose(0, 3, 2, 1, 4)
            ).reshape(NE, NMC, 128, CK * 256)
            # [NE, 16, 128, 2*1024]: W2c[e][128(2kk+k2)+p, c] -> [e, kk, p, (k2 c)]
            w2h = np.ascontiguousarray(
                W2c.reshape(NE, HK // 2, 2, 128, 1024).transpose(0, 1, 3, 2, 4)
            ).reshape(NE, HK // 2, 128, 2 * 1024)
            # [128, NE*HK]: b1all[p, e*HK+hh] = b1[e, 128hh+p]
            b1t = np.ascontiguousarray(
                b1c.reshape(NE, HK, 128).transpose(2, 0, 1)).reshape(128, NE * HK)
            b2rw = np.ascontiguousarray(b2c.reshape(NE, CK, 128)).reshape(1, -1)
            es = np.zeros((E_R, 4, 128), f)
            for j in range(4):
                es[4 * g + j, j, :] = 1.0
            es = es.reshape(E_R, 4 * 128)
            group_cache[g] = (w1h, w2h, b1t, b2rw, es)
        w1h, w2h, b1t, b2rw, es = group_cache[g]
        # [128, CK*TOKC]: uTq[p, 512k+t] = u2[512q+t, 128k+p]
        uTq = np.ascontiguousarray(
            u2[TOKC * q:TOKC * (q + 1)].T.reshape(CK, 128, TOKC).transpose(1, 0, 2)
        ).reshape(128, CK * TOKC)
        in_maps.append({
            "uT": uTq, "uTr": uTq,
            "w1": w1h, "w2": w2h, "b1t": b1t, "b2r": b2rw,
            "cen": cenT, "esel": es,
        })
    return in_maps, u2


def _run(in_maps, trace=False):
    if "nc" not in _CACHE:
        _CACHE["nc"] = _build_program()
    nc = _CACHE["nc"]
    res = bass_utils.run_bass_kernel_spmd(
        nc, in_maps, core_ids=list(range(NCORES)), trace=trace)
    return res


def kernel(**inputs):
    in_maps, u2 = _prep_inputs(**inputs)
    trace = bool(int(os.environ.get("MOE_TRACE", "0")))
    res = _run(in_maps, trace=trace)
    _CACHE["last_results"] = res
    out2 = np.empty((B * T, C), np.float32)
    for q in range(4):
        part = (res.results[q]["outT"] + res.results[q + 4]["outT"])
        part = part.reshape(128, CK, TOKC).transpose(1, 0, 2).reshape(C, TOKC)
        out2[TOKC * q:TOKC * (q + 1)] = part.T + u2[TOKC * q:TOKC * (q + 1)]
    return out2.reshape(B, T, C)



# revision 3
# speedup vs baseline: 3.2244x; 3.2244x over previous
"""DeepSeekMoE layer on 8 Trainium2 NeuronCores — sparse expert dispatch.

Problem (hardcoded): B=2, T=1024, C=1024, H=4096, E_routed=8 (top-2 sigmoid
gating), E_shared=2, fp32 reference; rel-L2 tolerance 2e-2.

Key idea vs the dense baseline: only compute the top-2 routed experts per
token (4096 token-expert passes globally instead of 16384), and run all
matmuls in bf16 (full PE rate at any moving width; halves weight DMA).

Sharding: 4-way expert-parallel x 2-way token-parallel.
  Core c (g = c//2 expert group, k = c%2 token half) owns routed experts
  {2g, 2g+1} gathered from token half k (1024 "pool" tokens), plus shared
  expert (g%2) on a 512-token quarter of its pool. The pool token order is
  permuted per-core so the shared quarter always occupies pool slots 0..511
  (keeps the program SPMD-uniform). Host sums the 4 per-half partials plus
  the residual.

Routing/gating and rmsnorm run on the host (tiny: 2048x8 gating, one
normalize); the host also builds one-hot gather matrices P [pool->cap] and
gate-weighted scatter matrices S [cap->pool] per (core, expert). On device,
gather/scatter are PE matmuls; the routed MLP2 output (C-major) is PE-
transposed to token-major so the scatter can contract over the capacity dim.

Device per-core program:
  shared MLP1 (xnT C-major moving, W1 stationary) -> gelu -> h_s
  shared MLP2 (h_s moving, W2 stationary, b2 seeded via K=1 matmul) -> y_cm
    -> PE transpose -> y_tok_s (token-major)
  per routed expert e: gather xg = XN_tok^T P_e -> MLP1 -> gelu -> MLP2
    -> y_cm -> PE transpose -> y_tok_r[e]
  output stripes [128 tok, 512 C]: identity-scatter of y_tok_s (quarter)
    + gate-scatter S_e^T y_tok_r[e] accumulated in PSUM -> evict -> DMA.

loop_m > 1 wraps the body in a hardware For_i loop (timing only).
"""
import contextlib
import os
import numpy as np
import ml_dtypes

import concourse.bass as bass
import concourse.tile as tile
from concourse import bacc, mybir
from concourse import bass_utils
from concourse.masks import make_identity

F32 = mybir.dt.float32
BF = mybir.dt.bfloat16
AF = mybir.ActivationFunctionType
BF_NP = ml_dtypes.bfloat16

B, T, C, H = 2, 1024, 1024, 4096
E_R, E_S, TOPK = 8, 2, 2
NTOK = B * T            # 2048
NCORES = 8
NPOOL = 1024            # tokens per core pool (half)
NQ = 512                # shared-quarter tokens per core
CK = C // 128           # 8
HK = H // 128           # 32
NBLK = HK // 4          # 8 w1/w2 stream blocks (4 h-chunks each)
EPS = 1.1920929e-07

_CACHE = {}
_STATE = {"cap": None}


# --------------------------------------------------------------------------
# device program
# --------------------------------------------------------------------------

def _build_program(loop_m=1, cap=None):
    if cap is None:
        cap = _STATE["cap"]
    assert cap is not None, "_prep_inputs must run before _build_program"
    jch = (cap + 127) // 128
    capp = 128 * jch

    nc = bacc.Bacc("TRN2", target_bir_lowering=False, debug=False,
                   enable_asserts=False)

    d = {}
    d["xn_tok"] = nc.dram_tensor("xn_tok", [128, 8 * 1024], BF, kind="ExternalInput").ap()
    d["xnT"] = nc.dram_tensor("xnT", [128, CK * NQ], BF, kind="ExternalInput").ap()
    d["w1s"] = nc.dram_tensor("w1s", [NBLK, 128, 4096], BF, kind="ExternalInput").ap()
    d["w2s"] = nc.dram_tensor("w2s", [NBLK, 128, 4096], BF, kind="ExternalInput").ap()
    d["w1r"] = nc.dram_tensor("w1r", [2, NBLK, 128, 4096], BF, kind="ExternalInput").ap()
    d["w2r"] = nc.dram_tensor("w2r", [2, NBLK, 128, 4096], BF, kind="ExternalInput").ap()
    d["pg"] = nc.dram_tensor("pg", [2, 128, 8 * cap], BF, kind="ExternalInput").ap()
    d["sg"] = nc.dram_tensor("sg", [2, 128, jch * 1024], BF, kind="ExternalInput").ap()
    d["b1"] = nc.dram_tensor("b1", [128, 3 * HK], F32, kind="ExternalInput").ap()
    d["b2r"] = nc.dram_tensor("b2r", [1, 2 * C], BF, kind="ExternalInput").ap()
    d["b2s"] = nc.dram_tensor("b2s", [1, C], BF, kind="ExternalInput").ap()
    d["out"] = nc.dram_tensor("outT", [128, 8 * 1024], F32, kind="ExternalOutput").ap()

    with tile.TileContext(nc) as tc:
        with (
            tc.tile_pool(name="cst", bufs=1) as cst,
            tc.tile_pool(name="io", bufs=1) as io,
            tc.tile_pool(name="hs", bufs=1) as hs,
            tc.tile_pool(name="hr", bufs=1) as hr,
            tc.tile_pool(name="yp", bufs=2) as yp,
            tc.tile_pool(name="ytr", bufs=1) as ytr,
            tc.tile_pool(name="w1p", bufs=2) as w1p,
            tc.tile_pool(name="w2p", bufs=2) as w2p,
            tc.tile_pool(name="ot", bufs=3) as ot,
            tc.tile_pool(name="pp", bufs=8, space="PSUM") as pp,
        ):
            pools = dict(cst=cst, io=io, hs=hs, hr=hr, yp=yp, ytr=ytr,
                         w1p=w1p, w2p=w2p, ot=ot, pp=pp)
            loop = tc.For_i(0, loop_m, 1) if loop_m > 1 else contextlib.nullcontext()
            with loop:
                _moe_body(nc, d, pools, cap, jch, capp)

    nc.compile()
    return nc


def _moe_body(nc, d, p, cap, jch, capp):
    cst, io, hs, hr, yp, ytr, w1p, w2p, ot, pp = (
        p["cst"], p["io"], p["hs"], p["hr"], p["yp"], p["ytr"],
        p["w1p"], p["w2p"], p["ot"], p["pp"])

    # ---- constants ----
    identf = cst.tile([128, 128], F32, tag="identf", name="identf")
    make_identity(nc, identf[:])
    ident = cst.tile([128, 128], BF, tag="ident", name="ident")
    nc.vector.tensor_copy(ident[:], identf[:])
    ones_f = cst.tile([1, 512], F32, tag="ones_f", name="ones_f")
    nc.gpsimd.memset(ones_f[:], 1.0)
    onesw = cst.tile([1, 512], BF, tag="onesw", name="onesw")
    nc.vector.tensor_copy(onesw[:], ones_f[:])
    onesc = cst.tile([1, cap], BF, tag="onesc", name="onesc")
    nc.vector.tensor_copy(onesc[:], ones_f[:, :cap])
    onesr = cst.tile([1, 128], BF, tag="onesr", name="onesr")
    nc.vector.tensor_copy(onesr[:], ones_f[:, :128])

    # ---- input loads ----
    xn_tok = io.tile([128, 8 * 1024], BF, tag="xn_tok", name="xn_tok")
    nc.sync.dma_start(xn_tok[:, :4096], d["xn_tok"][:, :4096])
    nc.sync.dma_start(xn_tok[:, 4096:], d["xn_tok"][:, 4096:])
    xnT = io.tile([128, CK * NQ], BF, tag="xnT", name="xnT")
    nc.sync.dma_start(xnT[:], d["xnT"])
    pg = []
    for e in range(2):
        pge = io.tile([128, 8 * cap], BF, tag=f"pg{e}", name=f"pg{e}")
        nc.sync.dma_start(pge[:], d["pg"][e])
        pg.append(pge)
    sg = []
    for e in range(2):
        sge = io.tile([128, jch * 1024], BF, tag=f"sg{e}", name=f"sg{e}")
        nc.sync.dma_start(sge[:], d["sg"][e])
        sg.append(sge)
    b1 = io.tile([128, 3 * HK], F32, tag="b1", name="b1")
    nc.sync.dma_start(b1[:], d["b1"])
    b2r = io.tile([1, 2 * C], BF, tag="b2r", name="b2r")
    nc.sync.dma_start(b2r[:], d["b2r"])
    b2s = io.tile([1, C], BF, tag="b2s", name="b2s")
    nc.sync.dma_start(b2s[:], d["b2s"])

    # ---- shared expert MLP1: h_s[hh] = gelu(W1s^T xnT + b1) ----
    h_s = []
    for blk in range(NBLK):
        w1t = w1p.tile([128, 4096], BF, tag="w1blk", name=f"w1s_{blk}")
        nc.sync.dma_start(w1t[:], d["w1s"][blk])
        for i in range(4):
            hh = 4 * blk + i
            ph = pp.tile([128, 512], F32, tag="pp", name=f"phs_{hh}")
            for k in range(CK):
                nc.tensor.matmul(
                    ph[:], w1t[:, 1024 * i + 128 * k:1024 * i + 128 * (k + 1)],
                    xnT[:, NQ * k:NQ * (k + 1)],
                    start=(k == 0), stop=(k == CK - 1))
            ht = hs.tile([128, 512], BF, tag=f"hs{hh}", name=f"hs_{hh}")
            nc.scalar.activation(ht[:], ph[:], AF.Gelu, bias=b1[:, hh:hh + 1])
            h_s.append(ht)

    # ---- shared expert MLP2 (C-major): py[k] = W2s^T h_s + b2s ----
    pys = [pp.tile([128, 512], F32, tag="pp", name=f"pys_{k}") for k in range(CK)]
    for k in range(CK):
        nc.tensor.matmul(pys[k][:], b2s[:, 128 * k:128 * (k + 1)], onesw[:],
                         start=True, stop=False)
    for blk in range(NBLK):
        w2t = w2p.tile([128, 4096], BF, tag="w2blk", name=f"w2s_{blk}")
        nc.sync.dma_start(w2t[:], d["w2s"][blk])
        for i in range(4):
            hh = 4 * blk + i
            for k in range(CK):
                nc.tensor.matmul(
                    pys[k][:], w2t[:, 1024 * i + 128 * k:1024 * i + 128 * (k + 1)],
                    h_s[hh][:], start=False, stop=(hh == HK - 1))
    ys_cm = []
    for k in range(CK):
        yc = hs.tile([128, 512], BF, tag=f"yscm{k}", name=f"yscm{k}")
        nc.vector.tensor_copy(yc[:], pys[k][:])
        ys_cm.append(yc)
    y_tok_s = []
    for j in range(4):
        ytp = pp.tile([128, 1024], BF, tag="pp", name=f"ytps{j}")
        for k in range(CK):
            nc.tensor.transpose(ytp[:, 128 * k:128 * (k + 1)],
                                ys_cm[k][:, 128 * j:128 * (j + 1)], ident[:])
        yts = hs.tile([128, 1024], BF, tag=f"ytoks{j}", name=f"ytoks{j}")
        nc.vector.tensor_copy(yts[:], ytp[:])
        y_tok_s.append(yts)

    # ---- routed experts ----
    y_tok_r = [[None] * jch for _ in range(2)]
    for e in range(2):
        # gather: xg[k] = xn_tok^T P_e  (C-major, cap tokens)
        xg = yp.tile([128, CK * cap], BF, tag="xg", name=f"xg{e}")
        for k in range(CK):
            gps = pp.tile([128, cap], F32, tag="pp", name=f"gps_{e}_{k}")
            for tch in range(8):
                nc.tensor.matmul(
                    gps[:], xn_tok[:, 1024 * tch + 128 * k:1024 * tch + 128 * (k + 1)],
                    pg[e][:, cap * tch:cap * (tch + 1)],
                    start=(tch == 0), stop=(tch == 7))
            nc.scalar.copy(xg[:, cap * k:cap * (k + 1)], gps[:])
        # MLP1
        h_r = []
        for blk in range(NBLK):
            w1t = w1p.tile([128, 4096], BF, tag="w1blk", name=f"w1r_{e}_{blk}")
            nc.sync.dma_start(w1t[:], d["w1r"][e, blk])
            for i in range(4):
                hh = 4 * blk + i
                ph = pp.tile([128, cap], F32, tag="pp", name=f"phr_{e}_{hh}")
                for k in range(CK):
                    nc.tensor.matmul(
                        ph[:], w1t[:, 1024 * i + 128 * k:1024 * i + 128 * (k + 1)],
                        xg[:, cap * k:cap * (k + 1)],
                        start=(k == 0), stop=(k == CK - 1))
                ht = hr.tile([128, cap], BF, tag=f"hr{hh}", name=f"hr_{e}_{hh}")
                nc.scalar.activation(ht[:], ph[:], AF.Gelu,
                                     bias=b1[:, HK * (1 + e) + hh:HK * (1 + e) + hh + 1])
                h_r.append(ht)
        # MLP2 (C-major) + b2 seed
        pyr = [pp.tile([128, cap], F32, tag="pp", name=f"pyr_{e}_{k}")
               for k in range(CK)]
        for k in range(CK):
            nc.tensor.matmul(pyr[k][:], b2r[:, C * e + 128 * k:C * e + 128 * (k + 1)],
                             onesc[:], start=True, stop=False)
        for blk in range(NBLK):
            w2t = w2p.tile([128, 4096], BF, tag="w2blk", name=f"w2r_{e}_{blk}")
            nc.sync.dma_start(w2t[:], d["w2r"][e, blk])
            for i in range(4):
                hh = 4 * blk + i
                for k in range(CK):
                    nc.tensor.matmul(
                        pyr[k][:], w2t[:, 1024 * i + 128 * k:1024 * i + 128 * (k + 1)],
                        h_r[hh][:], start=False, stop=(hh == HK - 1))
        # evict (zero-padded to capp), transpose to token-major
        for k in range(CK):
            yc = yp.tile([128, capp], BF, tag=f"yrcm{k}", name=f"yrcm_{e}_{k}")
            nc.vector.tensor_copy(yc[:, :cap], pyr[k][:])
            if capp > cap:
                nc.gpsimd.memset(yc[:, cap:], 0.0)
            if k == 0:
                yr_cm = []
            yr_cm.append(yc)
        for j in range(jch):
            ytp = pp.tile([128, 1024], BF, tag="pp", name=f"ytpr_{e}_{j}")
            for k in range(CK):
                nc.tensor.transpose(ytp[:, 128 * k:128 * (k + 1)],
                                    yr_cm[k][:, 128 * j:128 * (j + 1)], ident[:])
            ytt = ytr.tile([128, 1024], BF, tag=f"ytr{e}_{j}", name=f"ytr_{e}_{j}")
            nc.vector.tensor_copy(ytt[:], ytp[:])
            y_tok_r[e][j] = ytt

    # ---- output stripes: residual-less partial = shared + routed scatters ----
    for tch in range(8):
        for half in range(2):
            acc = pp.tile([128, 512], F32, tag="pp", name=f"acc_{tch}_{half}")
            ops = []
            if tch < 4:
                ops.append(("sh",))
            for e in range(2):
                for j in range(jch):
                    ops.append(("rt", e, j))
            for idx, op in enumerate(ops):
                st, sp = (idx == 0), (idx == len(ops) - 1)
                if op[0] == "sh":
                    nc.tensor.matmul(acc[:], ident[:],
                                     y_tok_s[tch][:, 512 * half:512 * (half + 1)],
                                     start=st, stop=sp)
                else:
                    _, e, j = op
                    nc.tensor.matmul(
                        acc[:], sg[e][:, 1024 * j + 128 * tch:1024 * j + 128 * (tch + 1)],
                        y_tok_r[e][j][:, 512 * half:512 * (half + 1)],
                        start=st, stop=sp)
            outt = ot.tile([128, 512], F32, tag="out", name=f"out_{tch}_{half}")
            nc.scalar.copy(outt[:], acc[:])
            nc.sync.dma_start(d["out"][:, 1024 * tch + 512 * half:1024 * tch + 512 * (half + 1)],
                              outt[:])


# --------------------------------------------------------------------------
# host prep
# --------------------------------------------------------------------------

def _routing(u2, centroids):
    """scores/top-2/normalized gates, matching the jax reference."""
    f = np.float32
    scores = 1.0 / (1.0 + np.exp(-(u2 @ np.asarray(centroids, f))))
    top2 = np.argsort(-scores, axis=1, kind="stable")[:, :TOPK]
    denom = scores.sum(axis=1, keepdims=True)
    gk = np.take_along_axis(scores, top2, axis=1) / denom
    gmat = np.zeros((NTOK, E_R), f)
    np.put_along_axis(gmat, top2, gk.astype(f), axis=1)
    return top2, gmat


def _pack_w1(w1):
    # [C, H] -> [NBLK, 128, 4096]: [blk, p, 1024*i + 128*k + j] = w1[128k+p, 128(4blk+i)+j]
    a = w1.reshape(CK, 128, NBLK, 4, 128).transpose(2, 1, 3, 0, 4)
    return np.ascontiguousarray(a).reshape(NBLK, 128, 4096)


def _pack_w2(w2):
    # [H, C] -> [NBLK, 128, 4096]: [blk, p, 1024*i + c] = w2[128(4blk+i)+p, c]
    a = w2.reshape(NBLK, 4, 128, C).transpose(0, 2, 1, 3)
    return np.ascontiguousarray(a).reshape(NBLK, 128, 4096)


def _prep_inputs(u, g_shared, W1_s, b1_s, W2_s, b2_s,
                 g_routed, W1_r, b1_r, W2_r, b2_r, centroids):
    f = np.float32
    u2 = np.ascontiguousarray(np.asarray(u, f).reshape(NTOK, C))
    rms = np.sqrt(np.mean(u2 * u2, axis=1, keepdims=True) + EPS)
    xn = (u2 / rms)
    gsh = np.asarray(g_shared, f).reshape(C, 1)
    grt = np.asarray(g_routed, f).reshape(C, 1)
    top2, gmat = _routing(u2, centroids)

    # per-(core, local expert) selections, global capacity
    sels = {}
    maxc = 1
    for c in range(NCORES):
        g, k = c // 2, c % 2
        hq = g // 2
        q_sh = 2 * k + hq
        q_ot = 2 * k + (1 - hq)
        pool_idx = np.concatenate([
            np.arange(NQ * q_sh, NQ * (q_sh + 1)),
            np.arange(NQ * q_ot, NQ * (q_ot + 1))])
        t2p = top2[pool_idx]
        for el in range(2):
            eg = 2 * g + el
            sel = np.nonzero((t2p == eg).any(axis=1))[0]
            sels[(c, el)] = (pool_idx, sel)
            maxc = max(maxc, len(sel))
    cap = ((maxc + 8 + 15) // 16) * 16
    assert cap <= 512, f"routed capacity {maxc} too imbalanced"
    jch = (cap + 127) // 128
    _STATE["cap"] = cap

    in_maps = []
    aux_pool = []
    group_cache = {}
    for c in range(NCORES):
        g, k = c // 2, c % 2
        s = g % 2
        if g not in group_cache:
            w1r = np.stack([_pack_w1((grt * np.asarray(W1_r[2 * g + el], f)))
                            for el in range(2)]).astype(BF_NP)
            w2r = np.stack([_pack_w2(np.asarray(W2_r[2 * g + el], f))
                            for el in range(2)]).astype(BF_NP)
            w1s = _pack_w1(gsh * np.asarray(W1_s[s], f)).astype(BF_NP)
            w2s = _pack_w2(np.asarray(W2_s[s], f)).astype(BF_NP)
            b1c = np.stack([np.asarray(b1_s[s], f)] +
                           [np.asarray(b1_r[2 * g + el], f) for el in range(2)])
            b1t = np.ascontiguousarray(
                b1c.reshape(3, HK, 128).transpose(2, 0, 1)).reshape(128, 3 * HK)
            b2rw = np.concatenate([np.asarray(b2_r[2 * g + el], f)
                                   for el in range(2)]).reshape(1, 2 * C).astype(BF_NP)
            b2sw = np.asarray(b2_s[s], f).reshape(1, C).astype(BF_NP)
            group_cache[g] = (w1r, w2r, w1s, w2s, b1t, b2rw, b2sw)
        w1r, w2r, w1s, w2s, b1t, b2rw, b2sw = group_cache[g]

        pool_idx, _ = sels[(c, 0)]
        xnp = xn[pool_idx]                                   # [1024, C] f32
        xn_tok = np.ascontiguousarray(
            xnp.reshape(8, 128, C).transpose(1, 0, 2)).reshape(128, 8 * C).astype(BF_NP)
        xnT = np.ascontiguousarray(
            xnp[:NQ].T.reshape(CK, 128, NQ).transpose(1, 0, 2)).reshape(128, CK * NQ).astype(BF_NP)

        pmat = np.zeros((2, NPOOL, cap), f)
        smat = np.zeros((2, jch * 128, NPOOL), f)
        for el in range(2):
            eg = 2 * g + el
            _, sel = sels[(c, el)]
            n = len(sel)
            pmat[el, sel, np.arange(n)] = 1.0
            smat[el, np.arange(n), sel] = gmat[pool_idx[sel], eg]
        pgm = np.ascontiguousarray(
            pmat.reshape(2, 8, 128, cap).transpose(0, 2, 1, 3)).reshape(2, 128, 8 * cap).astype(BF_NP)
        sgm = np.ascontiguousarray(
            smat.reshape(2, jch, 128, NPOOL).transpose(0, 2, 1, 3)).reshape(2, 128, jch * NPOOL).astype(BF_NP)

        in_maps.append({
            "xn_tok": xn_tok, "xnT": xnT,
            "w1s": w1s, "w2s": w2s, "w1r": w1r, "w2r": w2r,
            "pg": pgm, "sg": sgm,
            "b1": b1t, "b2r": b2rw, "b2s": b2sw,
        })
        aux_pool.append(pool_idx)
    return in_maps, (u2, aux_pool)


def _run(in_maps, trace=False):
    cap = _STATE["cap"]
    key = ("nc", cap)
    if key not in _CACHE:
        _CACHE[key] = _build_program(cap=cap)
    nc = _CACHE[key]
    res = bass_utils.run_bass_kernel_spmd(
        nc, in_maps, core_ids=list(range(NCORES)), trace=trace)
    return res


def kernel(**inputs):
    in_maps, (u2, aux_pool) = _prep_inputs(**inputs)
    trace = bool(int(os.environ.get("MOE_TRACE", "0")))
    res = _run(in_maps, trace=trace)
    _CACHE["last_results"] = res
    out2 = u2.astype(np.float64)
    for c in range(NCORES):
        part = res.results[c]["outT"].reshape(128, 8, 1024).transpose(1, 0, 2).reshape(NPOOL, C)
        out2[aux_pool[c]] += part.astype(np.float64)
    return out2.astype(np.float32).reshape(B, T, C)
